# revision 1
# baseline (speedup 1.0000x reference)
"""Trainium2 Bass kernel: 3x3 stride-1 pad-1 conv2d, N=16,Cin=64,Cout=128,H=W=224.

Sharding: data-parallel over batch: 8 cores x 2 images each.

Per-core algorithm:
  - x lives in SBUF bands of R output rows per image at W+1=225 row stride:
    band row i = image row y0-1+i; element 224 of each row is a zero pad
    column (plus a zero guard element before row 0). With flat-shifted
    taps, out(y,0)'s dc=0 tap reads the previous row's pad and
    out(y,223)'s dc=2 tap reads its own row's pad -- both zero, so border
    columns come out exactly right with no fix-up pass.
    partitions 0-63 = img0 channels, 64-127 = img1 channels.
  - conv = sum over 9 taps (dr,dc) of fp16 matmuls:
      psum[co, 2 rows] += w[ci, tap, co].T @ band[ci, rows 2c+dr.., cols dc-1..]
    K=64 (Cin) partitions, M=128 (Cout), N=448 (2 output rows, one PSUM
    bank). fp16 in, fp22 multiply, fp32 accumulate; x is cast to fp16 on
    the host (halves input HBM traffic), weights are fp16.
  - img0 matmuls use PE rows 0-63, img1 rows 64-127 (tile_position derived
    from base partitions) -> the two streams run concurrently in disjoint
    row-groups of the systolic array (~107 ns per matmul sustained).
  - PSUM chunk [128, 448] evicted to SBUF staging with fused bias add
    (DVE 3/4, ACT 1/4); staged band DMA'd out on the scalar-engine queue
    so input loads (sync queue) and output stores overlap.
"""

import numpy as np

N_IMG, C_IN, C_OUT, KS, H, W = 16, 64, 128, 3, 224, 224
N_CORES = 8
IMGS_PER_CORE = N_IMG // N_CORES  # 2
R = 32  # output rows per band
WP = W + 1  # padded row stride in SBUF
TAPS = [(dr, dc) for dr in range(KS) for dc in range(KS)]


def build_conv_program(h=H, w=W, r=R, evict_split=3, out_bf16=False):
    import concourse.bacc as bacc
    import concourse.mybir as mybir
    import concourse.tile as tile

    wp = w + 1
    n_bands = h // r
    n_chunk = r // 2
    chunk = 2 * w  # 448
    flat = r * w
    assert h % r == 0 and r % 2 == 0
    # 1 guard elem (acts as row -1's pad), (r+2) rows of wp, 1 tail guard
    band_len = 1 + (r + 2) * wp + 1
    f32 = mybir.dt.float32
    f16 = mybir.dt.float16
    odt = mybir.dt.bfloat16 if out_bf16 else f32

    nc = bacc.Bacc("TRN2", target_bir_lowering=False)

    x_d = nc.dram_tensor("x", [IMGS_PER_CORE, C_IN, h, w], f16, kind="ExternalInput")
    w_d = nc.dram_tensor("w", [C_IN, 9, C_OUT], f16, kind="ExternalInput")
    b_d = nc.dram_tensor("bias", [C_OUT, 1], f32, kind="ExternalInput")
    zz_d = nc.dram_tensor("zz", [128, w], f16, kind="ExternalInput")
    out_d = nc.dram_tensor(
        "out", [IMGS_PER_CORE, C_OUT, h, w], odt, kind="ExternalOutput"
    )

    with tile.TileContext(nc) as tc:
        with (
            tc.tile_pool(name="const", bufs=1) as const_pool,
            tc.tile_pool(name="xband", bufs=2) as x_pool,
            tc.tile_pool(name="outs", bufs=2) as o_pool,
            tc.tile_pool(name="psum", bufs=8, space="PSUM") as p_pool,
        ):
            # fp16 weights: the per-matmul LDWEIGHTS hides under the N=448
            # moving stream. PE upconverts to fp22; accumulation is fp32.
            w_sb = const_pool.tile([128, 9, C_OUT], f16)
            nc.sync.dma_start(out=w_sb[0:64], in_=w_d[:])
            nc.sync.dma_start(out=w_sb[64:128], in_=w_d[:])
            bias_sb = const_pool.tile([C_OUT, 1], f32)
            nc.sync.dma_start(out=bias_sb[:], in_=b_d[:])

            bands = [
                x_pool.tile([128, band_len], f16, tag="band", name=f"band{i}")
                for i in range(2)
            ]
            for bt in bands:
                # zero the pad column of every row + the leading guard elem;
                # interior DMAs never touch these, so once is enough.
                nc.vector.memset(bt[:, 0 : 1 + (r + 2) * wp : wp], 0.0)

            for b in range(n_bands):
                y0 = b * r
                bt = bands[b % 2]
                bv = bt[:, 1 : 1 + (r + 2) * wp].rearrange(
                    "p (a c) -> p a c", c=wp
                )
                rows_lo = max(y0 - 1, 0)
                rows_hi = min(y0 + r + 1, h)
                dst_r0 = rows_lo - (y0 - 1)
                nrows = rows_hi - rows_lo
                if b == 0:
                    # top halo row of the image is zero
                    nc.sync.dma_start(out=bv[:, 0, 0:w], in_=zz_d[:])
                if b == n_bands - 1:
                    # bottom halo row is zero (buffer may hold stale data)
                    nc.sync.dma_start(out=bv[:, r + 1, 0:w], in_=zz_d[:])
                n_pieces = 4 if b == 0 else 1
                for img in range(IMGS_PER_CORE):
                    p0 = img * 64
                    for pc in range(n_pieces):
                        r_a = pc * nrows // n_pieces
                        r_b = (pc + 1) * nrows // n_pieces
                        nc.sync.dma_start(
                            out=bv[p0 : p0 + 64, dst_r0 + r_a : dst_r0 + r_b, 0:w],
                            in_=x_d[img, :, rows_lo + r_a : rows_lo + r_b, :],
                        )

                ost = [
                    o_pool.tile(
                        [C_OUT, flat], odt, tag=f"ost{img}", name=f"ost{img}_{b}"
                    )
                    for img in range(IMGS_PER_CORE)
                ]

                for c in range(n_chunk):
                    ps = [
                        p_pool.tile(
                            [C_OUT, chunk],
                            f32,
                            tag="ps",
                            bufs=8,
                            name=f"ps{i}_{b}_{c}",
                        )
                        for i in range(2)
                    ]
                    for t, (dr, dc) in enumerate(TAPS):
                        st = t == 0
                        sp = t == 8
                        base = 1 + (2 * c + dr) * wp + dc - 1
                        for img in range(IMGS_PER_CORE):
                            p0 = img * 64
                            rhs = bt[p0 : p0 + 64, base : base + 2 * wp].rearrange(
                                "p (a c) -> p a c", c=wp
                            )[:, :, 0:w]
                            nc.tensor.matmul(
                                ps[img][:],
                                w_sb[p0 : p0 + 64, t, :],
                                rhs,
                                start=st,
                                stop=sp,
                            )
                    for img in range(IMGS_PER_CORE):
                        dst = ost[img][:, c * chunk : (c + 1) * chunk]
                        if (c % 4) < evict_split:
                            nc.vector.tensor_scalar_add(dst, ps[img][:], bias_sb[:])
                        else:
                            nc.scalar.add(dst, ps[img][:], bias_sb[:])

                # Split stores so the final piece overlaps trailing evictions.
                n_out = 2
                for img in range(IMGS_PER_CORE):
                    for oc in range(n_out):
                        r_a = oc * r // n_out
                        r_b = (oc + 1) * r // n_out
                        nc.scalar.dma_start(
                            out=out_d[img, :, y0 + r_a : y0 + r_b, :],
                            in_=ost[img][:, r_a * w : r_b * w],
                        )

    nc.compile()
    return nc


def prep_weight(weight: np.ndarray) -> np.ndarray:
    # [C_OUT, C_IN, 3, 3] -> [C_IN, 9, C_OUT]
    return np.ascontiguousarray(weight.transpose(1, 2, 3, 0).reshape(C_IN, 9, C_OUT))


def run_conv(x, weight, bias, trace=False, h=H, r=R, out_bf16=False, evict_split=3):
    """x [16,64,224,224] f32. Returns (out [16,128,224,224] f32, results)."""
    from concourse.bass_utils import run_bass_kernel_spmd

    x = np.asarray(x, dtype=np.float32).astype(np.float16)
    w_t = prep_weight(np.asarray(weight, dtype=np.float32)).astype(np.float16)
    b_t = np.ascontiguousarray(np.asarray(bias, dtype=np.float32).reshape(C_OUT, 1))

    nc = build_conv_program(h=h, r=r, out_bf16=out_bf16, evict_split=evict_split)
    zz_np = np.zeros((128, W), np.float16)
    in_maps = [
        {
            "x": np.ascontiguousarray(x[i * IMGS_PER_CORE : (i + 1) * IMGS_PER_CORE]),
            "w": w_t,
            "bias": b_t,
            "zz": zz_np,
        }
        for i in range(N_CORES)
    ]
    res = run_bass_kernel_spmd(nc, in_maps, core_ids=list(range(N_CORES)), trace=trace)
    out = np.concatenate([r_["out"] for r_ in res.results], axis=0)
    if out.dtype != np.float32:
        out = out.astype(np.float32)
    return out, res


def kernel(**inputs) -> np.ndarray:
    out, _ = run_conv(inputs["x"], inputs["weight"], inputs["bias"])
    return out



# revision 2
# speedup vs baseline: 1.3606x; 1.3606x over previous
"""Trainium2 Bass kernel: 3x3 stride-1 pad-1 conv2d, N=16,Cin=64,Cout=128,H=W=224.

Sharding: data-parallel over batch: 8 cores x 2 images each.

v3 design (vs v1 baseline at ~244us):
  - Inputs are host-padded to [226, 225] (zero halo row top+bottom, zero pad
    column right). A band of 56 output rows needs input rows y0-1..y0+56 =
    58 padded rows, which is ONE fully contiguous DMA per image per band
    (~1.67 MB, max DMA efficiency). The padded column doubles as the conv
    zero-padding: with flat-shifted taps, tap value for flat out-elem e is
    band[e + 225*dr + dc], and all out-of-image reads land on zero pads.
  - Output kept in the same 225-stride flat layout (out elem (y,x) at
    225*y+x; col 224 is a garbage pad column, stripped on the host). This
    makes each chunk of output a pure 1D slice, so the matmul moving dim
    can be the PSUM-bank maximum N=512 instead of 448. Since each matmul
    carries an unavoidable ~107ns 128-column LDWEIGHTS (per-matmul weight
    reload is forced by the toolchain; walrus --enable-ldw-opt is
    incompatible with bass LDWEIGHTS) and two images' matmuls run
    concurrently in disjoint PE row halves, the slot rate is weight-load
    bound at ~220ns; N=512 amortizes that over 14% more output columns.
  - fp16 x and weights (PE fp22 multiply, fp32 accumulate), bf16 output
    (halves store traffic; rel err ~2e-3 vs 2e-2 budget).
  - 3 band buffers: band b+2's load WARs against band b-1's matmuls, giving
    the load two full bands of slack -> no PE stalls, no HAM re-throttle.
  - PSUM eviction with fused bias add: img0 on DVE, img1 on ACT, in
    parallel every chunk.
  - Stores: img0 bands on the sync queue, img1 bands on the scalar queue,
    4 pieces per (img, band) so stores start while later chunks evict.
"""

import numpy as np

N_IMG, C_IN, C_OUT, KS, H, W = 16, 64, 128, 3, 224, 224
N_CORES = 8
IMGS_PER_CORE = N_IMG // N_CORES  # 2
WP = W + 1  # 225: padded row stride
R = 56  # output rows per band
N_BANDS = H // R  # 4
BROWS = R + 2  # input rows per band (with halo)
FLAT = R * WP  # 12600 flat out elems per band (225-stride)
NCH = 512  # chunk size (PSUM bank max for f32)
TAPS = [(dr, dc) for dr in range(KS) for dc in range(KS)]


def build_conv_program(out_bf16=True):
    import concourse.bacc as bacc
    import concourse.mybir as mybir
    import concourse.tile as tile

    f32 = mybir.dt.float32
    f16 = mybir.dt.float16
    odt = mybir.dt.bfloat16 if out_bf16 else f32

    band_len = 1 + BROWS * WP + 1  # guard + 58 rows + tail guard = 13052
    chunks = []
    off = 0
    while off < FLAT:
        n = min(NCH, FLAT - off)
        chunks.append((off, n))
        off += n
    n_chunk = len(chunks)  # 25: 24 x 512 + 1 x 312

    nc = bacc.Bacc("TRN2", target_bir_lowering=False)

    x_d = nc.dram_tensor(
        "x", [IMGS_PER_CORE, C_IN, H + 2, WP], f16, kind="ExternalInput"
    )
    w_d = nc.dram_tensor("w", [128, 9, C_OUT], f16, kind="ExternalInput")
    b_d = nc.dram_tensor("bias", [C_OUT, 1], f32, kind="ExternalInput")
    out_d = nc.dram_tensor(
        "out", [IMGS_PER_CORE, C_OUT, N_BANDS, FLAT], odt, kind="ExternalOutput"
    )

    with tile.TileContext(nc) as tc:
        with (
            tc.tile_pool(name="const", bufs=1) as const_pool,
            tc.tile_pool(name="xband", bufs=3) as x_pool,
            tc.tile_pool(name="outs", bufs=2) as o_pool,
            tc.tile_pool(name="psum", bufs=8, space="PSUM") as p_pool,
        ):
            w_sb = const_pool.tile([128, 9, C_OUT], f16, name="w_sb")
            nc.sync.dma_start(out=w_sb[:], in_=w_d[:])
            bias_sb = const_pool.tile([C_OUT, 1], f32, name="bias_sb")
            nc.scalar.dma_start(out=bias_sb[:], in_=b_d[:])

            bands = [
                x_pool.tile([128, band_len], f16, tag="band", name=f"band{i}")
                for i in range(3)
            ]
            for bt in bands:
                # leading guard elem: the one conv read not covered by the
                # host-side zero pads. Never overwritten by band loads.
                nc.vector.memset(bt[:, 0:1], 0.0)

            for b in range(N_BANDS):
                bt = bands[b % 3]
                bv = bt[:, 1 : 1 + BROWS * WP].rearrange("p (a c) -> p a c", c=WP)
                # band b covers padded input rows 56b .. 56b+58
                n_pieces = 4 if b == 0 else 1
                for img in range(IMGS_PER_CORE):
                    p0 = img * 64
                    q = nc.sync if img == 0 else (nc.scalar if b == 0 else nc.sync)
                    for pc in range(n_pieces):
                        r_a = pc * BROWS // n_pieces
                        r_b = (pc + 1) * BROWS // n_pieces
                        q.dma_start(
                            out=bv[p0 : p0 + 64, r_a:r_b, :],
                            in_=x_d[img, :, R * b + r_a : R * b + r_b, :],
                        )

                ost = [
                    o_pool.tile(
                        [C_OUT, FLAT], odt, tag=f"ost{img}", name=f"ost{img}_{b}"
                    )
                    for img in range(IMGS_PER_CORE)
                ]

                for ci, (off, n) in enumerate(chunks):
                    ps = [
                        p_pool.tile(
                            [C_OUT, n], f32, tag="ps", bufs=8, name=f"ps{i}_{b}_{ci}"
                        )
                        for i in range(2)
                    ]
                    for t, (dr, dc) in enumerate(TAPS):
                        st = t == 0
                        sp = t == 8
                        base = off + WP * dr + dc
                        for img in range(IMGS_PER_CORE):
                            p0 = img * 64
                            nc.tensor.matmul(
                                ps[img][:],
                                w_sb[p0 : p0 + 64, t, :],
                                bt[p0 : p0 + 64, base : base + n],
                                start=st,
                                stop=sp,
                            )
                    dst0 = ost[0][:, off : off + n]
                    nc.vector.tensor_scalar_add(dst0, ps[0][:], bias_sb[:])
                    dst1 = ost[1][:, off : off + n]
                    nc.scalar.add(dst1, ps[1][:], bias_sb[:])

                n_out = 4
                for img in range(IMGS_PER_CORE):
                    q = nc.sync if img == 0 else nc.scalar
                    for oc in range(n_out):
                        e_a = oc * FLAT // n_out
                        e_b = (oc + 1) * FLAT // n_out
                        q.dma_start(
                            out=out_d[img, :, b, e_a:e_b],
                            in_=ost[img][:, e_a:e_b],
                        )

    nc.compile()
    return nc


def prep_weight(weight: np.ndarray) -> np.ndarray:
    # [C_OUT, C_IN, 3, 3] -> [C_IN, 9, C_OUT] duplicated into both
    # partition halves -> [128, 9, C_OUT]
    wt = np.ascontiguousarray(
        weight.transpose(1, 2, 3, 0).reshape(C_IN, 9, C_OUT)
    )
    return np.concatenate([wt, wt], axis=0)


def run_conv(x, weight, bias, trace=False, out_bf16=True):
    """x [16,64,224,224] f32. Returns (out [16,128,224,224] f32, results)."""
    from concourse.bass_utils import run_bass_kernel_spmd

    x = np.asarray(x, dtype=np.float32).astype(np.float16)
    xp = np.zeros((N_IMG, C_IN, H + 2, WP), np.float16)
    xp[:, :, 1 : H + 1, 0:W] = x
    w_t = prep_weight(np.asarray(weight, dtype=np.float32)).astype(np.float16)
    b_t = np.ascontiguousarray(
        np.asarray(bias, dtype=np.float32).reshape(C_OUT, 1)
    )

    nc = build_conv_program(out_bf16=out_bf16)
    in_maps = [
        {
            "x": np.ascontiguousarray(
                xp[i * IMGS_PER_CORE : (i + 1) * IMGS_PER_CORE]
            ),
            "w": w_t,
            "bias": b_t,
        }
        for i in range(N_CORES)
    ]
    res = run_bass_kernel_spmd(nc, in_maps, core_ids=list(range(N_CORES)), trace=trace)
    # [2,128,4,12600] per core -> [16,128,4,56,225] -> strip pad col
    out = np.concatenate([r_["out"] for r_ in res.results], axis=0)
    out = out.reshape(N_IMG, C_OUT, N_BANDS, R, WP)[:, :, :, :, 0:W]
    out = np.ascontiguousarray(out.reshape(N_IMG, C_OUT, H, W))
    if out.dtype != np.float32:
        out = out.astype(np.float32)
    return out, res


def kernel(**inputs) -> np.ndarray:
    out, _ = run_conv(inputs["x"], inputs["weight"], inputs["bias"])
    return out


# revision 4
# speedup vs baseline: 1.3612x; 1.0004x over previous
"""Trainium2 Bass kernel: 3x3 stride-1 pad-1 conv2d, N=16,Cin=64,Cout=128,H=W=224.

Sharding: data-parallel over batch: 8 cores x 2 images each.

v3 design (vs v1 baseline at ~244us):
  - Inputs are host-padded to [226, 225] (zero halo row top+bottom, zero pad
    column right). A band of 56 output rows needs input rows y0-1..y0+56 =
    58 padded rows, which is ONE fully contiguous DMA per image per band
    (~1.67 MB, max DMA efficiency). The padded column doubles as the conv
    zero-padding: with flat-shifted taps, tap value for flat out-elem e is
    band[e + 225*dr + dc], and all out-of-image reads land on zero pads.
  - Output kept in the same 225-stride flat layout (out elem (y,x) at
    225*y+x; col 224 is a garbage pad column, stripped on the host). This
    makes each chunk of output a pure 1D slice, so the matmul moving dim
    can be the PSUM-bank maximum N=512 instead of 448. Since each matmul
    carries an unavoidable ~107ns 128-column LDWEIGHTS (per-matmul weight
    reload is forced by the toolchain; walrus --enable-ldw-opt is
    incompatible with bass LDWEIGHTS) and two images' matmuls run
    concurrently in disjoint PE row halves, the slot rate is weight-load
    bound at ~220ns; N=512 amortizes that over 14% more output columns.
  - fp16 x and weights (PE fp22 multiply, fp32 accumulate), bf16 output
    (halves store traffic; rel err ~2e-3 vs 2e-2 budget).
  - 3 band buffers: band b+2's load WARs against band b-1's matmuls, giving
    the load two full bands of slack -> no PE stalls, no HAM re-throttle.
  - PSUM eviction with fused bias add: img0 on DVE, img1 on ACT, in
    parallel every chunk.
  - Stores: img0 bands on the sync queue, img1 bands on the scalar queue,
    4 pieces per (img, band) so stores start while later chunks evict.
"""

import numpy as np

N_IMG, C_IN, C_OUT, KS, H, W = 16, 64, 128, 3, 224, 224
N_CORES = 8
IMGS_PER_CORE = N_IMG // N_CORES  # 2
WP = W + 1  # 225: padded row stride
R = 56  # output rows per band
N_BANDS = H // R  # 4
BROWS = R + 2  # input rows per band (with halo)
FLAT = R * WP  # 12600 flat out elems per band (225-stride)
NCH = 512  # chunk size (PSUM bank max for f32)
TAPS = [(dr, dc) for dr in range(KS) for dc in range(KS)]


def build_conv_program(out_bf16=True):
    import concourse.bacc as bacc
    import concourse.mybir as mybir
    import concourse.tile as tile

    f32 = mybir.dt.float32
    f16 = mybir.dt.float16
    odt = mybir.dt.bfloat16 if out_bf16 else f32

    band_len = 1 + BROWS * WP + 1  # guard + 58 rows + tail guard = 13052
    chunks = []
    off = 0
    while off < FLAT:
        n = min(NCH, FLAT - off)
        chunks.append((off, n))
        off += n
    n_chunk = len(chunks)  # 25: 24 x 512 + 1 x 312

    nc = bacc.Bacc("TRN2", target_bir_lowering=False)

    x_d = nc.dram_tensor(
        "x", [IMGS_PER_CORE, C_IN, H + 2, WP], f16, kind="ExternalInput"
    )
    w_d = nc.dram_tensor("w", [128, 9, C_OUT], f16, kind="ExternalInput")
    b_d = nc.dram_tensor("bias", [C_OUT, 1], f32, kind="ExternalInput")
    out_d = nc.dram_tensor(
        "out", [IMGS_PER_CORE, C_OUT, N_BANDS, FLAT], odt, kind="ExternalOutput"
    )

    with tile.TileContext(nc) as tc:
        with (
            tc.tile_pool(name="const", bufs=1) as const_pool,
            tc.tile_pool(name="xband", bufs=3) as x_pool,
            tc.tile_pool(name="outs", bufs=2) as o_pool,
            tc.tile_pool(name="psum", bufs=8, space="PSUM") as p_pool,
        ):
            w_sb = const_pool.tile([128, 9, C_OUT], f16, name="w_sb")
            nc.sync.dma_start(out=w_sb[:], in_=w_d[:])
            bias_sb = const_pool.tile([C_OUT, 1], f32, name="bias_sb")
            nc.scalar.dma_start(out=bias_sb[:], in_=b_d[:])

            bands = [
                x_pool.tile([128, band_len], f16, tag="band", name=f"band{i}")
                for i in range(3)
            ]
            for bt in bands:
                # leading guard elem: the one conv read not covered by the
                # host-side zero pads. Never overwritten by band loads.
                nc.vector.memset(bt[:, 0:1], 0.0)

            for b in range(N_BANDS):
                bt = bands[b % 3]
                bv = bt[:, 1 : 1 + BROWS * WP].rearrange("p (a c) -> p a c", c=WP)
                # band b covers padded input rows 56b .. 56b+58. Band 0 is
                # loaded in pieces with a tiny first piece so the first
                # matmuls start as soon as possible; img0 on the sync queue,
                # img1 on the scalar queue so their issue overlaps.
                pieces = [(0, 5), (5, 23), (23, 41), (41, BROWS)] if b == 0 else [
                    (0, BROWS)
                ]
                for img in range(IMGS_PER_CORE):
                    p0 = img * 64
                    q = nc.sync if img == 0 else (nc.scalar if b == 0 else nc.sync)
                    for r_a, r_b in pieces:
                        q.dma_start(
                            out=bv[p0 : p0 + 64, r_a:r_b, :],
                            in_=x_d[img, :, R * b + r_a : R * b + r_b, :],
                        )

                ost = [
                    o_pool.tile(
                        [C_OUT, FLAT], odt, tag=f"ost{img}", name=f"ost{img}_{b}"
                    )
                    for img in range(IMGS_PER_CORE)
                ]

                for ci, (off, n) in enumerate(chunks):
                    ps = [
                        p_pool.tile(
                            [C_OUT, n], f32, tag="ps", bufs=8, name=f"ps{i}_{b}_{ci}"
                        )
                        for i in range(2)
                    ]
                    for t, (dr, dc) in enumerate(TAPS):
                        st = t == 0
                        sp = t == 8
                        base = off + WP * dr + dc
                        for img in range(IMGS_PER_CORE):
                            p0 = img * 64
                            nc.tensor.matmul(
                                ps[img][:],
                                w_sb[p0 : p0 + 64, t, :],
                                bt[p0 : p0 + 64, base : base + n],
                                start=st,
                                stop=sp,
                            )
                    dst0 = ost[0][:, off : off + n]
                    nc.vector.tensor_scalar_add(dst0, ps[0][:], bias_sb[:])
                    dst1 = ost[1][:, off : off + n]
                    nc.scalar.add(dst1, ps[1][:], bias_sb[:])

                # Finer store pieces on the last band shrink the kernel tail
                # (the final piece's transfer+completion is all that remains
                # after the last matmul).
                n_out = 8 if b == N_BANDS - 1 else 4
                for img in range(IMGS_PER_CORE):
                    q = nc.sync if img == 0 else nc.scalar
                    for oc in range(n_out):
                        e_a = oc * FLAT // n_out
                        e_b = (oc + 1) * FLAT // n_out
                        q.dma_start(
                            out=out_d[img, :, b, e_a:e_b],
                            in_=ost[img][:, e_a:e_b],
                        )

    nc.compile()
    return nc


def prep_weight(weight: np.ndarray) -> np.ndarray:
    # [C_OUT, C_IN, 3, 3] -> [C_IN, 9, C_OUT] duplicated into both
    # partition halves -> [128, 9, C_OUT]
    wt = np.ascontiguousarray(
        weight.transpose(1, 2, 3, 0).reshape(C_IN, 9, C_OUT)
    )
    return np.concatenate([wt, wt], axis=0)


def run_conv(x, weight, bias, trace=False, out_bf16=True):
    """x [16,64,224,224] f32. Returns (out [16,128,224,224] f32, results)."""
    from concourse.bass_utils import run_bass_kernel_spmd

    x = np.asarray(x, dtype=np.float32).astype(np.float16)
    xp = np.zeros((N_IMG, C_IN, H + 2, WP), np.float16)
    xp[:, :, 1 : H + 1, 0:W] = x
    w_t = prep_weight(np.asarray(weight, dtype=np.float32)).astype(np.float16)
    b_t = np.ascontiguousarray(
        np.asarray(bias, dtype=np.float32).reshape(C_OUT, 1)
    )

    nc = build_conv_program(out_bf16=out_bf16)
    in_maps = [
        {
            "x": np.ascontiguousarray(
                xp[i * IMGS_PER_CORE : (i + 1) * IMGS_PER_CORE]
            ),
            "w": w_t,
            "bias": b_t,
        }
        for i in range(N_CORES)
    ]
    res = run_bass_kernel_spmd(nc, in_maps, core_ids=list(range(N_CORES)), trace=trace)
    # [2,128,4,12600] per core -> [16,128,4,56,225] -> strip pad col
    out = np.concatenate([r_["out"] for r_ in res.results], axis=0)
    out = out.reshape(N_IMG, C_OUT, N_BANDS, R, WP)[:, :, :, :, 0:W]
    out = np.ascontiguousarray(out.reshape(N_IMG, C_OUT, H, W))
    if out.dtype != np.float32:
        out = out.astype(np.float32)
    return out, res


def kernel(**inputs) -> np.ndarray:
    out, _ = run_conv(inputs["x"], inputs["weight"], inputs["bias"])
    return out


# revision 5
# speedup vs baseline: 1.3965x; 1.0260x over previous
"""Trainium2 Bass kernel: 3x3 stride-1 pad-1 conv2d, N=16,Cin=64,Cout=128,H=W=224.

Sharding: data-parallel over batch: 8 cores x 2 images each.

v3 design (vs v1 baseline at ~244us):
  - Inputs are host-padded to [226, 225] (zero halo row top+bottom, zero pad
    column right). A band of 56 output rows needs input rows y0-1..y0+56 =
    58 padded rows, which is ONE fully contiguous DMA per image per band
    (~1.67 MB, max DMA efficiency). The padded column doubles as the conv
    zero-padding: with flat-shifted taps, tap value for flat out-elem e is
    band[e + 225*dr + dc], and all out-of-image reads land on zero pads.
  - Output kept in the same 225-stride flat layout (out elem (y,x) at
    225*y+x; col 224 is a garbage pad column, stripped on the host). This
    makes each chunk of output a pure 1D slice, so the matmul moving dim
    can be the PSUM-bank maximum N=512 instead of 448. Since each matmul
    carries an unavoidable ~107ns 128-column LDWEIGHTS (per-matmul weight
    reload is forced by the toolchain; walrus --enable-ldw-opt is
    incompatible with bass LDWEIGHTS) and two images' matmuls run
    concurrently in disjoint PE row halves, the slot rate is weight-load
    bound at ~220ns; N=512 amortizes that over 14% more output columns.
  - fp16 x and weights (PE fp22 multiply, fp32 accumulate), bf16 output
    (halves store traffic; rel err ~2e-3 vs 2e-2 budget).
  - 3 band buffers: band b+2's load WARs against band b-1's matmuls, giving
    the load two full bands of slack -> no PE stalls, no HAM re-throttle.
  - PSUM eviction with fused bias add: img0 on DVE, img1 on ACT, in
    parallel every chunk.
  - Stores: img0 bands on the sync queue, img1 bands on the scalar queue,
    4 pieces per (img, band) so stores start while later chunks evict.
"""

import numpy as np

N_IMG, C_IN, C_OUT, KS, H, W = 16, 64, 128, 3, 224, 224
N_CORES = 8
IMGS_PER_CORE = N_IMG // N_CORES  # 2
WP = W + 1  # 225: padded row stride
R = 56  # output rows per band
N_BANDS = H // R  # 4
BROWS = R + 2  # input rows per band (with halo)
FLAT = R * WP  # 12600 flat out elems per band (225-stride)
NCH = 512  # chunk size (PSUM bank max for f32)
TAPS = [(dr, dc) for dr in range(KS) for dc in range(KS)]


def build_conv_program(out_bf16=True):
    import concourse.bacc as bacc
    import concourse.mybir as mybir
    import concourse.tile as tile

    f32 = mybir.dt.float32
    f16 = mybir.dt.float16
    odt = mybir.dt.bfloat16 if out_bf16 else f32

    band_len = 1 + BROWS * WP + 1  # guard + 58 rows + tail guard = 13052
    chunks = []
    off = 0
    while off < FLAT:
        n = min(NCH, FLAT - off)
        chunks.append((off, n))
        off += n
    n_chunk = len(chunks)  # 25: 24 x 512 + 1 x 312

    nc = bacc.Bacc("TRN2", target_bir_lowering=False)

    x_d = nc.dram_tensor(
        "x", [IMGS_PER_CORE, C_IN, H + 2, WP], f16, kind="ExternalInput"
    )
    w_d = nc.dram_tensor("w", [128, 9, C_OUT], f16, kind="ExternalInput")
    b_d = nc.dram_tensor("bias", [C_OUT, 1], f32, kind="ExternalInput")
    out_d = nc.dram_tensor(
        "out", [IMGS_PER_CORE, C_OUT, N_BANDS, FLAT], odt, kind="ExternalOutput"
    )

    with tile.TileContext(nc) as tc:
        with (
            tc.tile_pool(name="const", bufs=1) as const_pool,
            tc.tile_pool(name="xband", bufs=3) as x_pool,
            tc.tile_pool(name="outs", bufs=2) as o_pool,
            tc.tile_pool(name="psum", bufs=8, space="PSUM") as p_pool,
        ):
            w_sb = const_pool.tile([128, 9, C_OUT], f16, name="w_sb")
            nc.sync.dma_start(out=w_sb[:], in_=w_d[:])
            bias_sb = const_pool.tile([C_OUT, 1], f32, name="bias_sb")
            nc.scalar.dma_start(out=bias_sb[:], in_=b_d[:])

            bands = [
                x_pool.tile([128, band_len], f16, tag="band", name=f"band{i}")
                for i in range(3)
            ]
            for bt in bands:
                # leading guard elem: the one conv read not covered by the
                # host-side zero pads. Never overwritten by band loads.
                nc.vector.memset(bt[:, 0:1], 0.0)

            for b in range(N_BANDS):
                bt = bands[b % 3]
                bv = bt[:, 1 : 1 + BROWS * WP].rearrange("p (a c) -> p a c", c=WP)
                # band b covers padded input rows 56b .. 56b+58. Both images
                # load in ONE 128-partition DMA (img dim merged into the
                # partition dim) for full DMA width. Band 0 uses graded
                # pieces so compute starts as soon as the first rows land.
                pieces = (
                    [(0, 5), (5, 11), (11, 18), (18, 26), (26, 36), (36, 47), (47, BROWS)]
                    if b == 0
                    else [(0, BROWS)]
                )
                for r_a, r_b in pieces:
                    nc.sync.dma_start(
                        out=bv[:, r_a:r_b, :],
                        in_=x_d[:, :, R * b + r_a : R * b + r_b, :].rearrange(
                            "i c r w -> (i c) r w"
                        ),
                    )

                ost = [
                    o_pool.tile(
                        [C_OUT, FLAT], odt, tag=f"ost{img}", name=f"ost{img}_{b}"
                    )
                    for img in range(IMGS_PER_CORE)
                ]

                for ci, (off, n) in enumerate(chunks):
                    ps = [
                        p_pool.tile(
                            [C_OUT, n], f32, tag="ps", bufs=8, name=f"ps{i}_{b}_{ci}"
                        )
                        for i in range(2)
                    ]
                    for t, (dr, dc) in enumerate(TAPS):
                        st = t == 0
                        sp = t == 8
                        base = off + WP * dr + dc
                        for img in range(IMGS_PER_CORE):
                            p0 = img * 64
                            nc.tensor.matmul(
                                ps[img][:],
                                w_sb[p0 : p0 + 64, t, :],
                                bt[p0 : p0 + 64, base : base + n],
                                start=st,
                                stop=sp,
                            )
                    dst0 = ost[0][:, off : off + n]
                    nc.vector.tensor_scalar_add(dst0, ps[0][:], bias_sb[:])
                    dst1 = ost[1][:, off : off + n]
                    nc.scalar.add(dst1, ps[1][:], bias_sb[:])

                # Finer store pieces on the last band shrink the kernel tail
                # (the final piece's transfer+completion is all that remains
                # after the last matmul).
                n_out = 8 if b == N_BANDS - 1 else 4
                for img in range(IMGS_PER_CORE):
                    q = nc.sync if img == 0 else nc.scalar
                    for oc in range(n_out):
                        e_a = oc * FLAT // n_out
                        e_b = (oc + 1) * FLAT // n_out
                        q.dma_start(
                            out=out_d[img, :, b, e_a:e_b],
                            in_=ost[img][:, e_a:e_b],
                        )

    nc.compile()
    return nc


def prep_weight(weight: np.ndarray) -> np.ndarray:
    # [C_OUT, C_IN, 3, 3] -> [C_IN, 9, C_OUT] duplicated into both
    # partition halves -> [128, 9, C_OUT]
    wt = np.ascontiguousarray(
        weight.transpose(1, 2, 3, 0).reshape(C_IN, 9, C_OUT)
    )
    return np.concatenate([wt, wt], axis=0)


def run_conv(x, weight, bias, trace=False, out_bf16=True):
    """x [16,64,224,224] f32. Returns (out [16,128,224,224] f32, results)."""
    from concourse.bass_utils import run_bass_kernel_spmd

    x = np.asarray(x, dtype=np.float32).astype(np.float16)
    xp = np.zeros((N_IMG, C_IN, H + 2, WP), np.float16)
    xp[:, :, 1 : H + 1, 0:W] = x
    w_t = prep_weight(np.asarray(weight, dtype=np.float32)).astype(np.float16)
    b_t = np.ascontiguousarray(
        np.asarray(bias, dtype=np.float32).reshape(C_OUT, 1)
    )

    nc = build_conv_program(out_bf16=out_bf16)
    in_maps = [
        {
            "x": np.ascontiguousarray(
                xp[i * IMGS_PER_CORE : (i + 1) * IMGS_PER_CORE]
            ),
            "w": w_t,
            "bias": b_t,
        }
        for i in range(N_CORES)
    ]
    res = run_bass_kernel_spmd(nc, in_maps, core_ids=list(range(N_CORES)), trace=trace)
    # [2,128,4,12600] per core -> [16,128,4,56,225] -> strip pad col
    out = np.concatenate([r_["out"] for r_ in res.results], axis=0)
    out = out.reshape(N_IMG, C_OUT, N_BANDS, R, WP)[:, :, :, :, 0:W]
    out = np.ascontiguousarray(out.reshape(N_IMG, C_OUT, H, W))
    if out.dtype != np.float32:
        out = out.astype(np.float32)
    return out, res


def kernel(**inputs) -> np.ndarray:
    out, _ = run_conv(inputs["x"], inputs["weight"], inputs["bias"])
    return out
